# revision 11
# baseline (speedup 1.0000x reference)
"""HSTU-style 4-layer transformer (B=8, T=2048, D=128, H=2) on 8 Trainium2 cores.

Data-parallel over batch: each NeuronCore runs one full sequence.
Residual stream kept feature-major [D=128 partitions, T=2048 free].
All matmuls in fp32r (full PE speed, ~1.5e-4 rel err); A-matrix in bf16.
RMSNorm rsqrt + 1/denom computed on DVE (quake seed + Newton; custom recip),
so the ScalarEngine only ever loads the Silu and Gelu table sets.
"""
import numpy as np
from contextlib import ExitStack

import concourse.bass as bass
import concourse.tile as tile
from concourse import bacc, mybir
from concourse._compat import with_exitstack
from concourse.alu_op_type import AluOpType
from concourse.masks import make_identity

F32 = mybir.dt.float32
F32R = mybir.dt.float32r
BF16 = mybir.dt.bfloat16
I32 = mybir.dt.int32
AF = mybir.ActivationFunctionType
MULT = AluOpType.mult
ADD = AluOpType.add

B, T, D, L, H = 8, 2048, 128, 4, 2
HD = D // H
NITEMS = 200000
EPS = 1e-8
SCALE = 1.0 / np.sqrt(HD)
NT = T // 512          # 4 t-chunks of 512
NS = T // 128          # 16 s-chunks of 128
QUAKE_C = 0x5F3759DF


def _quake_rsqrt(nc, pool, v, out_dtype, tag, prow=None):
    """1/sqrt(v) elementwise on DVE: quake seed + 2 Newton iterations.
    v: AP over partitions prow (or all), fp32 SBUF, strictly positive.
    Internal tiles are [128, n]; ops run on the prow slice so all operands
    share a base partition. Returns the final [128, n] tile (valid at prow)."""
    n = v.shape[-1]
    if prow is None:
        prow = slice(0, 128)
    q1 = pool.tile([128, n], I32, tag=f"{tag}_q1")
    nc.vector.tensor_scalar(out=q1[prow, :], in0=v.bitcast(I32), scalar1=1.0,
                            scalar2=None, op0=AluOpType.logical_shift_right)
    q2 = pool.tile([128, n], I32, tag=f"{tag}_q2")
    nc.vector.tensor_scalar(out=q2[prow, :], in0=q1[prow, :], scalar1=-1.0,
                            scalar2=float(QUAKE_C), op0=MULT, op1=ADD)
    cur = q2.bitcast(F32)
    for it in range(2):
        sq = pool.tile([128, n], F32, tag=f"{tag}_sq{it}")
        nc.vector.tensor_tensor(sq[prow, :], cur[prow, :], cur[prow, :], op=MULT)
        hv = pool.tile([128, n], F32, tag=f"{tag}_hv{it}")
        nc.vector.scalar_tensor_tensor(out=hv[prow, :], in0=v, scalar=-0.5,
                                       in1=sq[prow, :], op0=MULT, op1=MULT)
        w_ = pool.tile([128, n], F32, tag=f"{tag}_w{it}")
        nc.vector.tensor_scalar(out=w_[prow, :], in0=hv[prow, :], scalar1=1.5,
                                scalar2=None, op0=ADD)
        nxt = pool.tile([128, n], out_dtype if it == 1 else F32, tag=f"{tag}_y{it}")
        nc.vector.tensor_tensor(nxt[prow, :], cur[prow, :], w_[prow, :], op=MULT)
        cur = nxt
    return cur


@with_exitstack
def _build(ctx: ExitStack, tc: tile.TileContext, io, vb_nonzero: bool):
    nc = tc.nc
    cst = ctx.enter_context(tc.tile_pool(name="cst", bufs=1))
    big = ctx.enter_context(tc.tile_pool(name="big", bufs=1))
    sA = ctx.enter_context(tc.tile_pool(name="sA", bufs=3))
    gat = ctx.enter_context(tc.tile_pool(name="gat", bufs=3))
    st = ctx.enter_context(tc.tile_pool(name="st", bufs=2))
    stg = ctx.enter_context(tc.tile_pool(name="stg", bufs=1))
    ps_S = ctx.enter_context(tc.tile_pool(name="ps_S", bufs=2, space="PSUM"))
    ps_av = ctx.enter_context(tc.tile_pool(name="ps_av", bufs=1, space="PSUM"))
    ps_b = ctx.enter_context(tc.tile_pool(name="ps_b", bufs=2, space="PSUM"))

    # ---- load constants / weights, make fp32r copies ----
    ident = cst.tile([128, 128], F32)
    make_identity(nc, ident)

    wr = {}
    for nm in ("wq", "wk", "wu", "wv", "wf2", "wc1", "wc2"):
        f32t = stg.tile([128, L * 128], F32, tag="wstage")
        nc.sync.dma_start(f32t.rearrange("p (l m) -> p l m", l=L), io[nm].rearrange("l k m -> k l m"))
        rt = cst.tile([128, L * 128], F32R, tag=f"{nm}_r")
        nc.vector.tensor_copy(rt, f32t)
        wr[nm] = rt

    sel2_f = cst.tile([2, 128], F32)
    nc.sync.dma_start(sel2_f, io["sel2"])
    sel2 = cst.tile([2, 128], F32R)
    nc.vector.tensor_copy(sel2, sel2_f)
    ones1_f = cst.tile([1, 128], F32)
    nc.sync.dma_start(ones1_f, io["ones1"])
    ones1 = cst.tile([1, 128], F32R)
    nc.vector.tensor_copy(ones1, ones1_f)
    onesc_f = cst.tile([128, 1], F32)
    nc.sync.dma_start(onesc_f, io["onesc"])
    onesc = cst.tile([128, 1], F32R)
    nc.vector.tensor_copy(onesc, onesc_f)
    ones2t_f = cst.tile([128, 2], F32)
    nc.sync.dma_start(ones2t_f, io["ones2t"])
    ones2t = cst.tile([128, 2], F32R)
    nc.vector.tensor_copy(ones2t, ones2t_f)

    madd_f = stg.tile([128, 4 * 512], F32, tag="wstage")
    nc.sync.dma_start(madd_f.rearrange("p (k m) -> p k m", k=4), io["madd"].rearrange("k p m -> p k m"))
    madd = cst.tile([128, 4 * 512], F32R)
    nc.vector.tensor_copy(madd, madd_f)
    identr = cst.tile([128, 128], F32R)
    nc.vector.tensor_copy(identr, ident)
    posT = cst.tile([128, T], F32)
    nc.sync.dma_start(posT, io["posT"])
    idx = cst.tile([128, NS], I32)
    nc.sync.dma_start(idx, io["idx"])
    emb_s = cst.tile([128, 1], F32)
    nc.sync.dma_start(emb_s, io["emb_s"])
    last_s = cst.tile([128, 1], F32)
    nc.sync.dma_start(last_s, io["last_s"])
    bcol = {}
    for nm in ("ub", "qb", "kb", "c1b", "f2b", "c2b"):
        bt = cst.tile([128, L], F32, tag=f"{nm}_t")
        nc.sync.dma_start(bt, io[nm].rearrange("l k -> k l"))
        bcol[nm] = bt
    if vb_nonzero:
        vbB = cst.tile([128, L * 128], F32, tag="vbB")
        nc.sync.dma_start(vbB.rearrange("p (l m) -> p l m", l=L), io["vbB"].rearrange("l p m -> p l m"))

    # ---- helper: per-token rms rstd of a feature-major tile ----
    def ln_rstd(x_sb, tag):
        """x_sb: [128, T] f32. Returns rstd_row [1, T] F32R sbuf tile."""
        pd = st.tile([128, 16], F32, tag="ln_pd")
        for j in range(NT):
            xsq = st.tile([128, 512], F32R, tag="ln_xsq")
            nc.vector.tensor_tensor(xsq, x_sb[:, j * 512:(j + 1) * 512],
                                    x_sb[:, j * 512:(j + 1) * 512], op=MULT)
            msq_ps = ps_b.tile([1, 512], F32, tag="pb")
            nc.tensor.matmul(msq_ps, onesc, xsq, start=True, stop=True)
            row = st.tile([1, 512], F32, tag="ln_row")
            nc.vector.tensor_copy(row, msq_ps)
            nc.sync.dma_start(pd[32 * j:32 * (j + 1), :], row)
        mi = st.tile([128, 16], F32, tag="ln_mi")
        nc.vector.tensor_scalar(out=mi, in0=pd, scalar1=1.0 / D, scalar2=EPS,
                                op0=MULT, op1=ADD)
        rs = _quake_rsqrt(nc, st, mi[:, :], F32R, "lnq")
        row_r = st.tile([1, T], F32R, tag="ln_rowr")
        nc.sync.dma_start(row_r, rs)
        return row_r

    def bcast_row(row_r, j, tag):
        """K=1 broadcast matmul: row [1, T] F32R slice cols j*512.. -> psum [128, 512]."""
        bp = ps_b.tile([128, 512], F32, tag="pb")
        nc.tensor.matmul(bp, ones1, row_r[:, j * 512:(j + 1) * 512],
                         start=True, stop=True)
        return bp

    # ================= embedding gather + transpose + pos =================
    e_sb = big.tile([128, T], F32, tag="e")
    for g in range(4):
        tr_ps = ps_b.tile([128, 512], F32, tag="pb")
        for c4 in range(4):
            c = 4 * g + c4
            tok = gat.tile([128, 128], F32, tag="tok")
            nc.gpsimd.indirect_dma_start(
                out=tok, out_offset=None, in_=io["itab"][:, :],
                in_offset=bass.IndirectOffsetOnAxis(ap=idx[:, c:c + 1], axis=0))
            nc.tensor.transpose(tr_ps[:, c4 * 128:(c4 + 1) * 128], tok, ident)
        nc.vector.tensor_tensor(e_sb[:, g * 512:(g + 1) * 512], tr_ps,
                                posT[:, g * 512:(g + 1) * 512], op=ADD)

    x_sb = big.tile([128, T], F32, tag="xA")
    er = ln_rstd(e_sb, "emb")
    for j in range(NT):
        bp = bcast_row(er, j, "emb")
        nc.vector.scalar_tensor_tensor(
            out=x_sb[:, j * 512:(j + 1) * 512], in0=bp, scalar=emb_s[:, 0:1],
            in1=e_sb[:, j * 512:(j + 1) * 512], op0=MULT, op1=MULT)

    # ================= layers =================
    for l in range(L):
        lw = slice(l * 128, (l + 1) * 128)

        # ---- ln1 + U/Q/K/V projections ----
        r1 = ln_rstd(x_sb, f"l{l}ln1")
        xn = big.tile([128, T], F32R, tag="xn")
        for j in range(NT):
            bp = bcast_row(r1, j, "ln1")
            nc.vector.tensor_tensor(xn[:, j * 512:(j + 1) * 512], bp,
                                    x_sb[:, j * 512:(j + 1) * 512], op=MULT)

        U = big.tile([128, T], F32, tag="U")
        Q = big.tile([128, T], F32R, tag="Q")
        K = big.tile([128, T], F32R, tag="K")
        for nm, dst in (("wu", U), ("wq", Q), ("wk", K)):
            bnm = {"wu": "ub", "wq": "qb", "wk": "kb"}[nm]
            for j in range(NT):
                jc = slice(j * 512, (j + 1) * 512)
                up = ps_b.tile([128, 512], F32, tag="pb")
                nc.tensor.matmul(up, wr[nm][:, lw], xn[:, jc], start=True, stop=True)
                nc.scalar.activation(dst[:, jc], up, AF.Silu,
                                     bias=bcol[bnm][:, l:l + 1], scale=1.0)

        # V: token-major, layout per s-chunk [V0(64) | ones | V1(64) | ones] = 130 cols
        v130 = big.tile([128, NS * 130], BF16, tag="v130")
        ones_ap = bass.AP(tensor=v130.tensor, offset=v130.offset + 64,
                          ap=[v130.ap[0], [130, NS], [65, 2], [1, 1]])
        nc.gpsimd.memset(ones_ap, 1.0)
        for g in range(4):
            vp = ps_b.tile([128, 512], F32, tag="pb")
            for c4 in range(4):
                c = 4 * g + c4
                nc.tensor.matmul(vp[:, c4 * 128:(c4 + 1) * 128],
                                 xn[:, c * 128:(c + 1) * 128], wr["wv"][:, lw],
                                 start=True, stop=True)
            if vb_nonzero:
                vb_ap = bass.AP(tensor=vbB.tensor, offset=vbB.offset + l * 128,
                                ap=[vbB.ap[0], [0, 4], [1, 128]])
                vtmp = st.tile([128, 512], F32, tag="vtmp")
                nc.vector.tensor_tensor(vtmp, vp, vb_ap, op=ADD)
                vsrc = vtmp
            else:
                vsrc = vp
            vraw = st.tile([128, 512], BF16, tag="vraw")
            nc.scalar.activation(vraw, vsrc, AF.Silu)
            dst = bass.AP(tensor=v130.tensor, offset=v130.offset + g * 4 * 130,
                          ap=[v130.ap[0], [130, 4], [65, 2], [1, 64]])
            src = bass.AP(tensor=vraw.tensor, offset=vraw.offset,
                          ap=[vraw.ap[0], [128, 4], [64, 2], [1, 64]])
            nc.gpsimd.tensor_copy(dst, src)

        # ---- attention (per-t-chunk pipelined with hstu norm + f2) ----
        AVU = big.tile([128, T], F32, tag="AVU")
        pd = st.tile([128, 64], F32, tag="hstu_pd")
        GGrow = st.tile([2, T], F32R, tag="GGrow")
        x2 = big.tile([128, T], F32, tag="x2")
        for j in range(NT):
            jc = slice(j * 512, (j + 1) * 512)
            avb = ps_av.tile([128, 1024], F32, tag="avb")
            nsc = 4 * (j + 1)
            for i in range(nsc):
                Sp = ps_S.tile([128, 1024], F32, tag="S")
                diag = i >= 4 * j
                nc.tensor.matmul(Sp[:, 0:512], K[0:64, i * 128:(i + 1) * 128],
                                 Q[0:64, jc], start=True, stop=not diag)
                nc.tensor.matmul(Sp[:, 512:1024], K[64:128, i * 128:(i + 1) * 128],
                                 Q[64:128, jc], start=True, stop=not diag)
                if diag:
                    k = i - 4 * j
                    mslc = madd[:, k * 512:(k + 1) * 512]
                    nc.tensor.matmul(Sp[:, 0:512], identr, mslc, start=False, stop=True)
                    nc.tensor.matmul(Sp[:, 512:1024], identr, mslc, start=False, stop=True)
                A = sA.tile([128, 1024], BF16, tag="A")
                nc.scalar.activation(A, Sp, AF.Silu, scale=SCALE)
                A2 = sA.tile([128, 1024], BF16, tag="A2")
                nc.vector.tensor_scalar_max(A2, A, 0.0)
                nc.tensor.matmul(avb[0:65, 0:512], v130[:, i * 130:i * 130 + 65],
                                 A2[:, 0:512], start=(i == 0), stop=(i == nsc - 1))
                nc.tensor.matmul(avb[0:65, 512:1024], v130[:, i * 130 + 65:i * 130 + 130],
                                 A2[:, 512:1024], start=(i == 0), stop=(i == nsc - 1))
            # drain AV + stats for this t-chunk
            nc.vector.tensor_tensor(AVU[0:64, jc], avb[0:64, 0:512], U[0:64, jc], op=MULT)
            nc.vector.tensor_tensor(AVU[64:128, jc], avb[0:64, 512:1024],
                                    U[64:128, jc], op=MULT)
            avsq = st.tile([128, 512], F32R, tag="avsq")
            nc.scalar.activation(avsq[0:64, :], avb[0:64, 0:512], AF.Square)
            nc.scalar.activation(avsq[64:128, :], avb[0:64, 512:1024], AF.Square)
            ssq_ps = ps_b.tile([2, 512], F32, tag="pb")
            nc.tensor.matmul(ssq_ps, ones2t, avsq, start=True, stop=True)
            drow = st.tile([1, 1024], F32, tag="drow")
            nc.vector.tensor_copy(drow, avb[64:65, :])
            sqr = st.tile([2, 512], F32, tag="sqr")
            nc.vector.tensor_copy(sqr, ssq_ps)
            p32 = slice(32 * j, 32 * (j + 1))
            nc.sync.dma_start(pd[p32, 0:16], drow[:, 0:512])
            nc.sync.dma_start(pd[p32, 16:32], drow[:, 512:1024])
            nc.sync.dma_start(pd[p32, 32:48], sqr[0:1, :])
            nc.sync.dma_start(pd[p32, 48:64], sqr[1:2, :])

            # hstu norm scales for this chunk (all tiles sliced at p32)
            de = st.tile([128, 32], F32, tag="hde")
            nc.vector.tensor_scalar(out=de[p32, :], in0=pd[p32, 0:32], scalar1=EPS,
                                    scalar2=None, op0=ADD)
            rr = st.tile([128, 32], F32, tag="hrr")
            scr = st.tile([128, 32], F32, tag="hscr")
            nc.vector.reciprocal_approx_accurate(rr[p32, :], de[p32, :],
                                                 scratch=scr[p32, :])
            r2 = st.tile([128, 32], F32, tag="hr2")
            nc.vector.tensor_tensor(r2[p32, :], rr[p32, :], rr[p32, :], op=MULT)
            uu = st.tile([128, 32], F32, tag="huu")
            nc.vector.tensor_tensor(uu[p32, :], r2[p32, :], pd[p32, 32:64], op=MULT)
            mm_ = st.tile([128, 16], F32, tag="hmm")
            nc.vector.tensor_tensor(mm_[p32, :], uu[p32, 0:16], uu[p32, 16:32], op=ADD)
            mi = st.tile([128, 16], F32, tag="hmi")
            nc.vector.tensor_scalar(out=mi[p32, :], in0=mm_[p32, :], scalar1=1.0 / D,
                                    scalar2=EPS, op0=MULT, op1=ADD)
            Rq = _quake_rsqrt(nc, st, mi[p32, :], F32, "hq", prow=p32)
            GG = st.tile([128, 32], F32R, tag="GG")
            nc.vector.tensor_tensor(GG[p32, 0:16], rr[p32, 0:16], Rq[p32, :], op=MULT)
            nc.vector.tensor_tensor(GG[p32, 16:32], rr[p32, 16:32], Rq[p32, :], op=MULT)
            nc.sync.dma_start(GGrow[0:1, jc], GG[p32, 0:16])
            nc.sync.dma_start(GGrow[1:2, jc], GG[p32, 16:32])

            # f2 + residual for this chunk
            gb = ps_b.tile([128, 512], F32, tag="pb")
            nc.tensor.matmul(gb, sel2, GGrow[:, jc], start=True, stop=True)
            P = st.tile([128, 512], F32R, tag="Pf2")
            nc.vector.tensor_tensor(P, gb, AVU[:, jc], op=MULT)
            yf = ps_b.tile([128, 512], F32, tag="pb")
            nc.tensor.matmul(yf, wr["wf2"][:, lw], P, start=True, stop=True)
            nc.vector.scalar_tensor_tensor(
                out=x2[:, jc], in0=yf, scalar=bcol["f2b"][:, l:l + 1],
                in1=x_sb[:, jc], op0=ADD, op1=ADD)

        # ---- ln2 + FFN ----
        r2row = ln_rstd(x2, f"l{l}ln2")
        hh = big.tile([128, T], F32R, tag="U")
        xn2 = big.tile([128, T], F32R, tag="xn")
        for j in range(NT):
            jc = slice(j * 512, (j + 1) * 512)
            bp = bcast_row(r2row, j, "ln2")
            nc.vector.tensor_tensor(xn2[:, jc], bp, x2[:, jc], op=MULT)
        for j in range(NT):
            jc = slice(j * 512, (j + 1) * 512)
            cp = ps_b.tile([128, 512], F32, tag="pb")
            nc.tensor.matmul(cp, wr["wc1"][:, lw], xn2[:, jc], start=True, stop=True)
            nc.scalar.activation(hh[:, jc], cp, AF.Gelu,
                                 bias=bcol["c1b"][:, l:l + 1], scale=1.0)
        x3 = big.tile([128, T], F32, tag="xB" if l % 2 == 0 else "xA")
        for j in range(NT):
            jc = slice(j * 512, (j + 1) * 512)
            c2p = ps_b.tile([128, 512], F32, tag="pb")
            nc.tensor.matmul(c2p, wr["wc2"][:, lw], hh[:, jc], start=True, stop=True)
            nc.vector.scalar_tensor_tensor(
                out=x3[:, jc], in0=c2p, scalar=bcol["c2b"][:, l:l + 1],
                in1=x2[:, jc], op0=ADD, op1=ADD)
        x_sb = x3

    # ================= final norm + output =================
    rf = ln_rstd(x_sb, "fin")
    o_sb = big.tile([128, T], F32, tag="e")
    for j in range(NT):
        jc = slice(j * 512, (j + 1) * 512)
        bp = bcast_row(rf, j, "fin")
        nc.vector.scalar_tensor_tensor(
            out=o_sb[:, jc], in0=bp, scalar=last_s[:, 0:1],
            in1=x_sb[:, jc], op0=MULT, op1=MULT)
    nc.sync.dma_start(io["out"], o_sb)


_CACHE = {}


def _get_nc(vb_nonzero: bool):
    key = vb_nonzero
    if key in _CACHE:
        return _CACHE[key]
    nc = bacc.Bacc("TRN2", target_bir_lowering=False, debug=False)
    io = {}
    def din(name, shape, dt=F32):
        io[name] = nc.dram_tensor(name, shape, dt, kind="ExternalInput").ap()
    din("idx", (128, NS), I32)
    din("itab", (NITEMS + 1, 128))
    din("posT", (128, T))
    for nm in ("wq", "wk", "wu", "wv", "wf2", "wc1", "wc2"):
        din(nm, (L, 128, 128))
    for nm in ("ub", "qb", "kb", "c1b", "f2b", "c2b"):
        din(nm, (L, 128))
    if vb_nonzero:
        din("vbB", (L, 128, 128))
    din("sel2", (2, 128))
    din("madd", (4, 128, 512))
    din("ones1", (1, 128))
    din("onesc", (128, 1))
    din("ones2t", (128, 2))
    din("emb_s", (128, 1))
    din("last_s", (128, 1))
    io["out"] = nc.dram_tensor("out", (128, T), F32, kind="ExternalOutput").ap()
    with tile.TileContext(nc) as t:
        _build(t, io, vb_nonzero)
    nc.compile()
    _CACHE[key] = nc
    return nc


def _prep_maps(inputs):
    f32 = lambda a: np.ascontiguousarray(np.asarray(a, dtype=np.float32))
    log_seqs = np.asarray(inputs["log_seqs"]).astype(np.int64)
    itab = f32(inputs["item_table"])
    posT = f32(np.asarray(inputs["pos_table"], dtype=np.float32)[1:T + 1].T)
    ln1 = f32(inputs["ln1_s"]); ln2 = f32(inputs["ln2_s"])
    hstu = f32(inputs["hstu_ln_s"])
    com = {
        "itab": itab, "posT": posT,
        "wq": f32(ln1[:, :, None] * np.asarray(inputs["Qw"], np.float32)),
        "wk": f32(ln1[:, :, None] * np.asarray(inputs["Kw"], np.float32)),
        "wu": f32(ln1[:, :, None] * np.asarray(inputs["Uw"], np.float32)),
        "wv": f32(ln1[:, :, None] * np.asarray(inputs["Vw"], np.float32)),
        "wf2": f32(hstu[:, :, None] * np.asarray(inputs["f2w"], np.float32)),
        "wc1": f32(ln2[:, :, None] * np.asarray(inputs["c1w"], np.float32)),
        "wc2": f32(inputs["c2w"]),
        "ub": f32(inputs["Ub"]), "qb": f32(inputs["Qb"]), "kb": f32(inputs["Kb"]),
        "c1b": f32(inputs["c1b"]), "f2b": f32(inputs["f2b"]), "c2b": f32(inputs["c2b"]),
        "emb_s": f32(np.asarray(inputs["emb_ln_s"], np.float32).reshape(128, 1)),
        "last_s": f32(np.asarray(inputs["last_ln_s"], np.float32).reshape(128, 1)),
    }
    sel2 = np.zeros((2, 128), np.float32)
    sel2[0, 0:64] = 1.0
    sel2[1, 64:128] = 1.0
    com["sel2"] = sel2
    com["ones1"] = np.ones((1, 128), np.float32)
    madd = np.zeros((4, 128, 512), np.float32)
    for k in range(4):
        off = 128 * k
        cs = np.arange(128)[:, None]
        ct = np.arange(512)[None, :]
        madd[k] = np.where(ct >= cs + off, 0.0, -30000.0)
    com["madd"] = madd
    com["onesc"] = np.ones((128, 1), np.float32)
    o2 = np.zeros((128, 2), np.float32)
    o2[0:64, 0] = 1.0
    o2[64:128, 1] = 1.0
    com["ones2t"] = o2
    vb = np.asarray(inputs["Vb"], np.float32)
    vb_nonzero = bool(np.any(vb != 0.0))
    if vb_nonzero:
        com["vbB"] = f32(np.broadcast_to(vb[:, None, :], (L, 128, 128)))
    maps = []
    for b in range(B):
        m = dict(com)
        m["idx"] = np.ascontiguousarray(
            log_seqs[b].reshape(NS, 128).T.astype(np.int32))
        maps.append(m)
    return maps, vb_nonzero


def kernel(**inputs):
    from concourse.bass_utils import run_bass_kernel_spmd
    maps, vb_nonzero = _prep_maps(inputs)
    nc = _get_nc(vb_nonzero)
    res = run_bass_kernel_spmd(nc, maps, core_ids=list(range(B)))
    out = np.stack([res.results[b]["out"].T for b in range(B)], axis=0)
    return np.ascontiguousarray(out.astype(np.float32))


if __name__ == "__main__":
    # compile-only smoke test
    nc = _get_nc(False)
    import tempfile
    from concourse.bass_utils import compile_bass_kernel
    print("NEFF:", compile_bass_kernel(nc, tempfile.mkdtemp(prefix="hstu_")))
